# revision 1
# baseline (speedup 1.0000x reference)
"""GQA attention (RoPE + ALiBi + causal) on 8 trn2 NeuronCores.

Sharding: core c -> batch b = c//4, kv-group g = c%4 (4 q-heads + 1 kv-head
per core, column-sharded Wq/Wk/Wv, row-sharded Wo; host sums the 4 partial
Wo outputs per batch).

All device matmuls run in float32r (full-rate PE) with N=512 moving dims.
Everything is kept transposed ([feature, token]) so softmax reductions over
keys become partition-dim reductions done with ones-vector matmuls, and the
per-key ALiBi column bias rides the exp() activation's per-partition bias.
The per-query ALiBi term is added with a K=1 ones matmul into the same PSUM
accumulation. Causal structure: only lower-triangle key tiles are computed;
diagonal tiles get the (transposed) mask block added before exp.
"""
import sys

if '/opt/trn_rl_repo' not in sys.path:
    sys.path.insert(0, '/opt/trn_rl_repo')

import numpy as np

B, T, D = 2, 2048, 2048
H, KV = 16, 4
HD = D // H          # 128
NREP = H // KV       # 4
KVD = 512            # per-core q width (4 heads x 128)
P = 128
TB = 512             # t-block
NBLK = T // TB       # 4
NC = D // P          # 16 contraction tiles
NJ = T // P          # 16 key tiles
ALIBI_W = 0.1
SCALE = (1.0 - ALIBI_W) / np.sqrt(np.float32(HD))

_cache = {}


def _build():
    from concourse import bacc, mybir
    from concourse.tile import TileContext

    F32 = mybir.dt.float32
    FR = mybir.dt.float32r
    EXP = mybir.ActivationFunctionType.Exp

    nc = bacc.Bacc()
    xT = nc.declare_dram_parameter("xT", [D, T], F32, isOutput=False)
    wq = nc.declare_dram_parameter("wq", [D, KVD], F32, isOutput=False)
    wk = nc.declare_dram_parameter("wk", [D, P], F32, isOutput=False)
    wv = nc.declare_dram_parameter("wv", [D, P], F32, isOutput=False)
    wo = nc.declare_dram_parameter("wo", [KVD, D], F32, isOutput=False)
    cosq = nc.declare_dram_parameter("cosq", [P, T], F32, isOutput=False)
    sinq = nc.declare_dram_parameter("sinq", [P, T], F32, isOutput=False)
    cosk = nc.declare_dram_parameter("cosk", [P, T], F32, isOutput=False)
    sink = nc.declare_dram_parameter("sink", [P, T], F32, isOutput=False)
    cb = nc.declare_dram_parameter("cb", [P, NREP * NBLK * NJ], F32, isOutput=False)
    maskT = nc.declare_dram_parameter("maskT", [P, 4 * TB], F32, isOutput=False)
    onesc = nc.declare_dram_parameter("onesc", [P, 1], F32, isOutput=False)
    idin = nc.declare_dram_parameter("idin", [P, P], F32, isOutput=False)
    out = nc.declare_dram_parameter("out", [T, D], F32, isOutput=True)

    with TileContext(nc) as tc:
        with (
            tc.tile_pool(name="const", bufs=1) as cpool,
            tc.tile_pool(name="kv", bufs=1) as kvpool,
            tc.tile_pool(name="tabs", bufs=1) as tpool,
            tc.tile_pool(name="xin", bufs=3) as xpool,
            tc.tile_pool(name="work", bufs=2) as wpool,
            tc.tile_pool(name="qt", bufs=4) as qpool,
            tc.tile_pool(name="pt", bufs=3) as ptpool,
            tc.tile_pool(name="ot", bufs=4) as opool,
            tc.tile_pool(name="ysb", bufs=2) as ypool,
            tc.tile_pool(name="small", bufs=2) as spool,
            tc.tile_pool(name="ps", bufs=1, space="PSUM") as pss,
        ):
            # ---- resident constants ----
            wq_sb = cpool.tile([P, NC, KVD], FR)
            wq_r = wq.rearrange("(c p) n -> p c n", p=P).bitcast(FR)
            for c in range(NC):
                nc.sync.dma_start(out=wq_sb[:, c], in_=wq_r[:, c])
            wk_sb = cpool.tile([P, NC, P], FR)
            wk_r = wk.rearrange("(c p) n -> p c n", p=P).bitcast(FR)
            wv_sb = cpool.tile([P, NC, P], FR)
            wv_r = wv.rearrange("(c p) n -> p c n", p=P).bitcast(FR)
            for c4 in range(4):
                nc.sync.dma_start(out=wk_sb[:, c4 * 4:(c4 + 1) * 4], in_=wk_r[:, c4 * 4:(c4 + 1) * 4])
                nc.sync.dma_start(out=wv_sb[:, c4 * 4:(c4 + 1) * 4], in_=wv_r[:, c4 * 4:(c4 + 1) * 4])
            wo_sb = cpool.tile([P, NREP, D], FR)
            wo_r = wo.rearrange("(h p) e -> p h e", p=P).bitcast(FR)
            for h in range(NREP):
                nc.sync.dma_start(out=wo_sb[:, h], in_=wo_r[:, h])
            cb_sb = cpool.tile([P, NREP * NBLK * NJ], F32)
            nc.sync.dma_start(out=cb_sb, in_=cb[:, :])
            maskT_sb = cpool.tile([P, 4 * TB], F32)
            nc.sync.dma_start(out=maskT_sb, in_=maskT[:, :])
            onesc_sb = cpool.tile([P, 1], FR)
            nc.sync.dma_start(out=onesc_sb, in_=onesc[:, :].bitcast(FR))
            id_sb = cpool.tile([P, P], FR)
            nc.sync.dma_start(out=id_sb, in_=idin[:, :].bitcast(FR))

            kT_sb = kvpool.tile([P, T], FR)          # roped K, [d, s]
            v_sb = kvpool.tile([P, NJ, P], FR)       # V tiles, [s, j, d']

            for bk in range(NBLK):
                t0 = bk * TB
                # ---- tables for this block ----
                cq_t = tpool.tile([P, TB], F32, tag="cq")
                nc.sync.dma_start(out=cq_t, in_=cosq[:, t0:t0 + TB])
                sq_t = tpool.tile([P, TB], F32, tag="sq")
                nc.sync.dma_start(out=sq_t, in_=sinq[:, t0:t0 + TB])
                ck_t = tpool.tile([P, TB], F32, tag="ck")
                nc.sync.dma_start(out=ck_t, in_=cosk[:, t0:t0 + TB])
                sk_t = tpool.tile([P, TB], F32, tag="sk")
                nc.sync.dma_start(out=sk_t, in_=sink[:, t0:t0 + TB])

                # ---- projections ----
                q_ps = [None] * NREP
                q_ps[0] = pss.tile([P, TB], F32, tag="big", bufs=7, name=f"qps{bk}_0")
                q_ps[1] = pss.tile([P, TB], F32, tag="big", bufs=7, name=f"qps{bk}_1")
                k_ps = pss.tile([P, TB], F32, tag="big", bufs=7, name=f"kps{bk}")
                v_ps = pss.tile([P, TB], F32, tag="big", bufs=7, name=f"vps{bk}")
                for c in range(NC):
                    xt = xpool.tile([P, TB], FR, tag="xt", name=f"xtA{bk}_{c}")
                    nc.sync.dma_start(out=xt, in_=xT[c * P:(c + 1) * P, t0:t0 + TB].bitcast(FR))
                    for h in (0, 1):
                        nc.tensor.matmul(q_ps[h], wq_sb[:, c, h * P:(h + 1) * P], xt,
                                         start=(c == 0), stop=(c == NC - 1))
                    nc.tensor.matmul(k_ps, wk_sb[:, c, :], xt, start=(c == 0), stop=(c == NC - 1))
                    nc.tensor.matmul(v_ps, wv_sb[:, c, :], xt, start=(c == 0), stop=(c == NC - 1))
                q_ps[2] = pss.tile([P, TB], F32, tag="big", bufs=7, name=f"qps{bk}_2")
                q_ps[3] = pss.tile([P, TB], F32, tag="big", bufs=7, name=f"qps{bk}_3")
                for c in range(NC):
                    xt = xpool.tile([P, TB], FR, tag="xt", name=f"xtB{bk}_{c}")
                    nc.sync.dma_start(out=xt, in_=xT[c * P:(c + 1) * P, t0:t0 + TB].bitcast(FR))
                    for h in (2, 3):
                        nc.tensor.matmul(q_ps[h], wq_sb[:, c, h * P:(h + 1) * P], xt,
                                         start=(c == 0), stop=(c == NC - 1))

                # ---- RoPE ----
                def rope(dst, src_ps, cos_t, sin_t, nm):
                    raw = wpool.tile([P, TB], F32, tag="raw", name=f"raw{nm}")
                    nc.scalar.copy(raw, src_ps)
                    swp = wpool.tile([P, TB], F32, tag="swp", name=f"swp{nm}")
                    nc.sync.dma_start(out=swp[0:64, :], in_=raw[64:128, :])
                    nc.sync.dma_start(out=swp[64:128, :], in_=raw[0:64, :])
                    m1 = wpool.tile([P, TB], F32, tag="m1", name=f"m1{nm}")
                    nc.vector.tensor_mul(m1, src_ps, cos_t)
                    m2 = wpool.tile([P, TB], F32, tag="m2", name=f"m2{nm}")
                    nc.vector.tensor_mul(m2, swp, sin_t)
                    nc.vector.tensor_add(dst, m1, m2)

                q_sb = []
                for h in range(NREP):
                    qh = qpool.tile([P, TB], FR, tag="qT", name=f"qT{bk}_{h}")
                    rope(qh, q_ps[h], cq_t, sq_t, f"q{bk}_{h}")
                    q_sb.append(qh)
                rope(kT_sb[:, t0:t0 + TB], k_ps, ck_t, sk_t, f"k{bk}")

                # ---- V: copy + transpose to [s, d'] ----
                vtmp = wpool.tile([P, TB], FR, tag="vtmp", name=f"vtmp{bk}")
                nc.scalar.copy(vtmp, v_ps)
                for sj in range(4):
                    vt_ps = pss.tile([P, P], FR, tag="big", bufs=7, name=f"vtps{bk}_{sj}")
                    nc.tensor.transpose(vt_ps, vtmp[:, sj * P:(sj + 1) * P], id_sb)
                    nc.vector.tensor_copy(v_sb[:, 4 * bk + sj, :], vt_ps)

                # ---- attention ----
                nj = 4 * bk + 4
                for h in range(NREP):
                    ot_ps = pss.tile([P, TB], F32, tag="big", bufs=7, name=f"otps{bk}_{h}")
                    cs_ps = pss.tile([1, TB], F32, tag="cs", bufs=1, name=f"csps{bk}_{h}")
                    for j in range(nj):
                        s_ps = pss.tile([P, TB], F32, tag="big", bufs=7, name=f"sps{bk}_{h}_{j}")
                        nc.tensor.matmul(s_ps, kT_sb[:, j * P:(j + 1) * P], q_sb[h],
                                         start=True, stop=True)
                        delta = j - 4 * bk
                        if delta >= 0:
                            nc.vector.tensor_add(s_ps, s_ps,
                                                 maskT_sb[:, delta * TB:(delta + 1) * TB])
                        pt = ptpool.tile([P, TB], FR, tag="pt", name=f"pt{bk}_{h}_{j}")
                        nc.scalar.activation(pt, s_ps, EXP,
                                             bias=cb_sb[:, (h * NBLK + bk) * NJ + j:(h * NBLK + bk) * NJ + j + 1])
                        nc.tensor.matmul(cs_ps, onesc_sb, pt,
                                         start=(j == 0), stop=(j == nj - 1))
                        nc.tensor.matmul(ot_ps, v_sb[:, j, :], pt,
                                         start=(j == 0), stop=(j == nj - 1))
                    rec = spool.tile([1, TB], F32, tag="rec", name=f"rec{bk}_{h}")
                    nc.vector.reciprocal(rec, cs_ps)
                    rbc = spool.tile([P, TB], F32, tag="rbc", name=f"rbc{bk}_{h}")
                    nc.gpsimd.partition_broadcast(rbc, rec)
                    oh = opool.tile([P, TB], FR, tag="ot", name=f"ot{bk}_{h}")
                    nc.vector.tensor_mul(oh, ot_ps, rbc)
                    q_sb[h] = oh  # reuse list slot to keep handles

                ot_sb = q_sb  # [h] -> [d', t] normalized attention out

                # ---- Wo partial ----
                for ts_ in range(4):
                    for e in range(4):
                        y_ps = pss.tile([P, TB], F32, tag="big", bufs=7, name=f"yps{bk}_{ts_}_{e}")
                        for h in range(NREP):
                            nc.tensor.matmul(y_ps, ot_sb[h][:, ts_ * P:(ts_ + 1) * P],
                                             wo_sb[:, h, e * TB:(e + 1) * TB],
                                             start=(h == 0), stop=(h == NREP - 1))
                        y_sb = ypool.tile([P, TB], F32, tag="ysb", name=f"y{bk}_{ts_}_{e}")
                        nc.vector.tensor_copy(y_sb, y_ps)
                        nc.sync.dma_start(
                            out=out[t0 + ts_ * P:t0 + (ts_ + 1) * P, e * TB:(e + 1) * TB],
                            in_=y_sb)

    nc.compile()
    return nc


def _prep_inputs(x, mask, freqs_cis, alibi_bias, Wq, Wk, Wv, Wo):
    """Host-side prep: transposes, RoPE tables, ALiBi bias decomposition."""
    f64 = np.float64
    idx = np.arange(HD)
    cos_full = freqs_cis[:, idx // 2]                     # [T, 128]
    sin_full = freqs_cis[:, (HD // 2) + idx // 2]         # [T, 128]
    sign = np.where(idx < HD // 2, -1.0, 1.0).astype(np.float32)
    cosT = np.ascontiguousarray(cos_full.T)               # [128, T]
    sinT_signed = np.ascontiguousarray((sin_full * sign[None, :]).T)

    cosq = (cosT * np.float32(SCALE)).astype(np.float32)
    sinq = (sinT_signed * np.float32(SCALE)).astype(np.float32)
    cosk = cosT.astype(np.float32)
    sink = sinT_signed.astype(np.float32)

    m = mask[0, 0]
    maskT = np.empty((P, 4 * TB), np.float32)
    for d in range(4):
        maskT[:, d * TB:(d + 1) * TB] = m[:TB, d * P:(d + 1) * P].T

    onesc = np.ones((P, 1), np.float32)
    idin = np.eye(P, dtype=np.float32)

    in_maps = []
    for c in range(8):
        b, g = c // 4, c % 4
        slopes = np.array([-f64(alibi_bias[0, g * NREP + hl, 1, 0]) for hl in range(NREP)])
        pvec = np.arange(P, dtype=f64)
        jvec = np.arange(NJ, dtype=f64)
        # cb[p, h, bk, j] = ALIBI_W*slope*(j*128 + p) - ALIBI_W*slope*(bk*512 + 511)
        bkvec = np.arange(NBLK, dtype=f64)
        cbv = (ALIBI_W * slopes[:, None, None, None]
               * (jvec[None, None, :, None] * P + pvec[None, None, None, :]
                  - (bkvec[None, :, None, None] * TB + (TB - 1))))
        cbm = np.ascontiguousarray(cbv.transpose(3, 0, 1, 2).reshape(P, NREP * NBLK * NJ)).astype(np.float32)
        in_maps.append({
            "xT": np.ascontiguousarray(x[b].T),
            "wq": np.ascontiguousarray(Wq[:, g * KVD:(g + 1) * KVD]),
            "wk": np.ascontiguousarray(Wk[:, g * P:(g + 1) * P]),
            "wv": np.ascontiguousarray(Wv[:, g * P:(g + 1) * P]),
            "wo": np.ascontiguousarray(Wo[g * KVD:(g + 1) * KVD, :]),
            "cosq": cosq, "sinq": sinq, "cosk": cosk, "sink": sink,
            "cb": cbm, "maskT": maskT,
            "onesc": onesc, "idin": idin,
        })
    return in_maps


def kernel(x, mask, freqs_cis, alibi_bias, Wq, Wk, Wv, Wo, _trace=False, _trace_kwargs=None):
    from concourse.bass_utils import run_bass_kernel_spmd

    if "nc" not in _cache:
        _cache["nc"] = _build()
    nc = _cache["nc"]

    in_maps = _prep_inputs(np.asarray(x, np.float32), np.asarray(mask, np.float32),
                           np.asarray(freqs_cis, np.float32), np.asarray(alibi_bias, np.float32),
                           np.asarray(Wq, np.float32), np.asarray(Wk, np.float32),
                           np.asarray(Wv, np.float32), np.asarray(Wo, np.float32))
    kw = {}
    if _trace:
        kw = dict(trace=True, **(_trace_kwargs or {}))
    res = run_bass_kernel_spmd(nc, in_maps, list(range(8)), **kw)

    full = np.zeros((B, T, D), np.float32)
    for c in range(8):
        full[c // 4] += res.results[c]["out"]
    if _trace:
        _cache["last_trace"] = res
    return full



# revision 8
# speedup vs baseline: 1.8918x; 1.8918x over previous
"""GQA attention (RoPE + ALiBi + causal) on 8 trn2 NeuronCores.

Sharding: core c -> batch b = c//4, kv-group g = c%4 (4 q-heads + 1 kv-head
per core, column-sharded Wq/Wk/Wv, row-sharded Wo; host sums the 4 partial
Wo outputs per batch).

v2: bf16 operands everywhere on SBUF (PSUM/scores/biases stay f32), one-pass
projections, per-c interleaved weight+x streaming at startup, rotate-half via
a PE permutation matmul (no SBUF-SBUF DMA), Wo of block bk-1 issued after
projections of bk so the PE stays busy during RoPE, software-pipelined
attention with 3-deep score lookahead, column-restricted diagonal tiles
(fully-masked key columns skipped), triangle-only mask adds, packed per-block
softmax denominators with reciprocal_approx_fast.
"""
import sys

if '/opt/trn_rl_repo' not in sys.path:
    sys.path.insert(0, '/opt/trn_rl_repo')

import numpy as np
import ml_dtypes

BF = ml_dtypes.bfloat16

B, T, D = 2, 2048, 2048
H, KV = 16, 4
HD = D // H          # 128
NREP = H // KV       # 4
KVD = 512            # per-core q width (4 heads x 128)
P = 128
TB = 512             # t-block
NBLK = T // TB       # 4
NC = D // P          # 16 contraction tiles
NJ = T // P          # 16 key tiles
ALIBI_W = 0.1
SCALE = (1.0 - ALIBI_W) / np.sqrt(np.float32(HD))

_cache = {}


def _build():
    from concourse import bacc, mybir
    from concourse.tile import TileContext

    F32 = mybir.dt.float32
    BF16 = mybir.dt.bfloat16
    EXP = mybir.ActivationFunctionType.Exp

    nc = bacc.Bacc()
    xT = nc.declare_dram_parameter("xT", [D, T], BF16, isOutput=False)
    wq = nc.declare_dram_parameter("wq", [D, KVD], BF16, isOutput=False)
    wk = nc.declare_dram_parameter("wk", [D, P], BF16, isOutput=False)
    wv = nc.declare_dram_parameter("wv", [D, P], BF16, isOutput=False)
    wo = nc.declare_dram_parameter("wo", [KVD, D], BF16, isOutput=False)
    cosq = nc.declare_dram_parameter("cosq", [P, T], BF16, isOutput=False)
    sinq = nc.declare_dram_parameter("sinq", [P, T], BF16, isOutput=False)
    cosk = nc.declare_dram_parameter("cosk", [P, T], BF16, isOutput=False)
    sink = nc.declare_dram_parameter("sink", [P, T], BF16, isOutput=False)
    cb = nc.declare_dram_parameter("cb", [P, NREP * NBLK * NJ], F32, isOutput=False)
    maskT = nc.declare_dram_parameter("maskT", [P, P], F32, isOutput=False)
    onesc = nc.declare_dram_parameter("onesc", [P, 1], BF16, isOutput=False)
    idin = nc.declare_dram_parameter("idin", [P, P], BF16, isOutput=False)
    permi = nc.declare_dram_parameter("permi", [P, P], BF16, isOutput=False)
    out = nc.declare_dram_parameter("out", [T, D], BF16, isOutput=True)

    with TileContext(nc) as tc:
        with (
            tc.tile_pool(name="const", bufs=1) as cpool,
            tc.tile_pool(name="kv", bufs=1) as kvpool,
            tc.tile_pool(name="xin", bufs=4) as xpool,
            tc.tile_pool(name="rp", bufs=3) as rpool,
            tc.tile_pool(name="qt", bufs=6) as qpool,
            tc.tile_pool(name="pt", bufs=5) as ptpool,
            tc.tile_pool(name="ot", bufs=5) as opool,
            tc.tile_pool(name="ysb", bufs=3) as ypool,
            tc.tile_pool(name="small", bufs=2) as spool,
            tc.tile_pool(name="ps", bufs=1, space="PSUM") as pss,
        ):
            # ---- small resident constants (tiny DMAs, scalar queue) ----
            cb_sb = cpool.tile([P, NREP * NBLK * NJ], F32)
            nc.scalar.dma_start(out=cb_sb, in_=cb[:, :])
            maskT_sb = cpool.tile([P, P], F32)
            nc.scalar.dma_start(out=maskT_sb, in_=maskT[:, :])
            onesc_sb = cpool.tile([P, 1], BF16)
            nc.scalar.dma_start(out=onesc_sb, in_=onesc[:, :])
            id_sb = cpool.tile([P, P], BF16)
            nc.scalar.dma_start(out=id_sb, in_=idin[:, :])
            perm_sb = cpool.tile([P, P], BF16)
            nc.scalar.dma_start(out=perm_sb, in_=permi[:, :])

            # ---- weight tiles: declared here, streamed per-c inside bk=0 ----
            wq_sb = cpool.tile([P, NC, KVD], BF16)
            wq_r = wq.rearrange("(c p) n -> p c n", p=P)
            wk_sb = cpool.tile([P, NC, P], BF16)
            wk_r = wk.rearrange("(c p) n -> p c n", p=P)
            wv_sb = cpool.tile([P, NC, P], BF16)
            wv_r = wv.rearrange("(c p) n -> p c n", p=P)
            wo_sb = cpool.tile([P, NREP, D], BF16)
            wo_r = wo.rearrange("(h p) e -> p h e", p=P)

            # rope tables (full T), loaded after block-0 weights
            cq_sb = cpool.tile([P, T], BF16)
            sq_sb = cpool.tile([P, T], BF16)
            ck_sb = cpool.tile([P, T], BF16)
            sk_sb = cpool.tile([P, T], BF16)

            kT_sb = kvpool.tile([P, T], BF16)        # roped K, [d, s]
            v_sb = kvpool.tile([P, NJ, P], BF16)     # V tiles, [s, j, d']

            ohs_prev = None
            t0_prev = 0

            def rope(dst, src_ps, cos_sl, sin_sl, nm):
                raw = rpool.tile([P, TB], BF16, tag="raw", name=f"raw{nm}")
                nc.any.tensor_copy(raw, src_ps)
                sw_ps = pss.tile([P, TB], F32, tag="big", bufs=6, name=f"sw{nm}")
                nc.tensor.matmul(sw_ps, perm_sb, raw, start=True, stop=True)
                m1 = rpool.tile([P, TB], BF16, tag="m1", name=f"m1{nm}")
                nc.vector.tensor_mul(m1, raw, cos_sl)
                m2 = rpool.tile([P, TB], BF16, tag="m2", name=f"m2{nm}")
                nc.vector.tensor_mul(m2, sw_ps, sin_sl)
                nc.vector.tensor_add(dst, m1, m2)

            for bk in range(NBLK):
                t0 = bk * TB
                # ---- projections: one pass, 6 PSUM banks ----
                q_ps = [pss.tile([P, TB], F32, tag="big", bufs=6, name=f"qps{bk}_{h}")
                        for h in range(NREP)]
                k_ps = pss.tile([P, TB], F32, tag="big", bufs=6, name=f"kps{bk}")
                v_ps = pss.tile([P, TB], F32, tag="big", bufs=6, name=f"vps{bk}")
                for c in range(NC):
                    if bk == 0:
                        nc.scalar.dma_start(out=wq_sb[:, c], in_=wq_r[:, c])
                        nc.scalar.dma_start(out=wk_sb[:, c], in_=wk_r[:, c])
                        nc.scalar.dma_start(out=wv_sb[:, c], in_=wv_r[:, c])
                    xt = xpool.tile([P, TB], BF16, tag="xt", name=f"xt{bk}_{c}")
                    nc.sync.dma_start(out=xt, in_=xT[c * P:(c + 1) * P, t0:t0 + TB])
                    st, sp = (c == 0), (c == NC - 1)
                    nc.tensor.matmul(k_ps, wk_sb[:, c, :], xt, start=st, stop=sp)
                    nc.tensor.matmul(v_ps, wv_sb[:, c, :], xt, start=st, stop=sp)
                    for h in range(NREP):
                        nc.tensor.matmul(q_ps[h], wq_sb[:, c, h * P:(h + 1) * P], xt,
                                         start=st, stop=sp)
                if bk == 0:
                    # background loads: rope tables then wo (needed later)
                    nc.scalar.dma_start(out=ck_sb, in_=cosk[:, :])
                    nc.scalar.dma_start(out=sk_sb, in_=sink[:, :])
                    nc.scalar.dma_start(out=cq_sb, in_=cosq[:, :])
                    nc.scalar.dma_start(out=sq_sb, in_=sinq[:, :])
                    for h in range(NREP):
                        nc.scalar.dma_start(out=wo_sb[:, h], in_=wo_r[:, h])

                # ---- rope k + q0 first so attention can start early ----
                rope(kT_sb[:, t0:t0 + TB], k_ps, ck_sb[:, t0:t0 + TB],
                     sk_sb[:, t0:t0 + TB], f"k{bk}")
                q_sb = [None] * NREP
                q_sb[0] = qpool.tile([P, TB], BF16, tag="qT", name=f"qT{bk}_0")
                rope(q_sb[0], q_ps[0], cq_sb[:, t0:t0 + TB], sq_sb[:, t0:t0 + TB],
                     f"q{bk}_0")

                # ---- V: copy + transpose to [s, d'] ----
                vtmp = rpool.tile([P, TB], BF16, tag="vtmp", name=f"vtmp{bk}")
                nc.any.tensor_copy(vtmp, v_ps)
                for sj in range(4):
                    vt_ps = pss.tile([P, P], BF16, tag="big", bufs=6, name=f"vtps{bk}_{sj}")
                    nc.tensor.transpose(vt_ps, vtmp[:, sj * P:(sj + 1) * P], id_sb)
                    nc.vector.tensor_copy(v_sb[:, 4 * bk + sj, :], vt_ps)

                for h in range(1, NREP):
                    q_sb[h] = qpool.tile([P, TB], BF16, tag="qT", name=f"qT{bk}_{h}")
                    rope(q_sb[h], q_ps[h], cq_sb[:, t0:t0 + TB], sq_sb[:, t0:t0 + TB],
                         f"q{bk}_{h}")

                # ---- Wo of previous block (fills PE while rope runs) ----
                if ohs_prev is not None:
                    for ts_ in range(4):
                        for e in range(4):
                            y_ps = pss.tile([P, TB], F32, tag="big", bufs=6,
                                            name=f"yps{bk}_{ts_}_{e}")
                            for h in range(NREP):
                                nc.tensor.matmul(
                                    y_ps, ohs_prev[h][:, ts_ * P:(ts_ + 1) * P],
                                    wo_sb[:, h, e * TB:(e + 1) * TB],
                                    start=(h == 0), stop=(h == NREP - 1))
                            y_sb = ypool.tile([P, TB], BF16, tag="ysb",
                                              name=f"y{bk}_{ts_}_{e}")
                            nc.vector.tensor_copy(y_sb, y_ps)
                            nc.gpsimd.dma_start(
                                out=out[t0_prev + ts_ * P:t0_prev + (ts_ + 1) * P,
                                        e * TB:(e + 1) * TB],
                                in_=y_sb)

                # ---- attention: software-pipelined over flat (h, j) ----
                nj = 4 * bk + 4
                flat = [(h, j) for h in range(NREP) for j in range(nj)]
                cs_ps = [pss.tile([1, TB], F32, tag="cs", bufs=2, name=f"cs{bk}_{h}")
                         for h in range(NREP)]
                ot_ps = [pss.tile([P, TB], F32, tag="big", bufs=6, name=f"otps{bk}_{h}")
                         for h in range(NREP)]
                ohs = [None] * NREP

                def issue_s(idx):
                    h, j = flat[idx]
                    dlt = j - 4 * bk
                    c0 = P * dlt if dlt > 0 else 0
                    s_ps = pss.tile([P, TB], F32, tag="big", bufs=6,
                                    name=f"sps{bk}_{h}_{j}")
                    nc.tensor.matmul(s_ps[:, c0:], kT_sb[:, j * P:(j + 1) * P],
                                     q_sb[h][:, c0:], start=True, stop=True)
                    if dlt >= 0:
                        nc.vector.tensor_add(s_ps[:, c0:c0 + P], s_ps[:, c0:c0 + P],
                                             maskT_sb)
                    pt = ptpool.tile([P, TB], BF16, tag="pt", name=f"pt{bk}_{h}_{j}")
                    col = (h * NBLK + bk) * NJ + j
                    nc.scalar.activation(pt[:, c0:], s_ps[:, c0:], EXP,
                                         bias=cb_sb[:, col:col + 1])
                    return pt, c0

                def issue_po(idx, pt, c0):
                    h, j = flat[idx]
                    nc.tensor.matmul(cs_ps[h][:, c0:], onesc_sb, pt[:, c0:],
                                     start=(j == 0), stop=(j == nj - 1),
                                     skip_group_check=True)
                    nc.tensor.matmul(ot_ps[h][:, c0:], v_sb[:, j, :], pt[:, c0:],
                                     start=(j == 0), stop=(j == nj - 1),
                                     skip_group_check=True)
                    if j == nj - 1:
                        rec = spool.tile([1, TB], F32, tag="rec", name=f"rec{bk}_{h}")
                        nc.vector.reciprocal_approx_fast(out=rec, in_=cs_ps[h])
                        rbc = spool.tile([P, TB], F32, tag="rbc", name=f"rbc{bk}_{h}")
                        nc.gpsimd.partition_broadcast(rbc, rec)
                        oh = opool.tile([P, TB], BF16, tag="oh", name=f"oh{bk}_{h}")
                        nc.vector.tensor_mul(oh, ot_ps[h], rbc)
                        ohs[h] = oh

                LOOK = 3
                pend = []
                for idx in range(min(LOOK, len(flat))):
                    pend.append(issue_s(idx))
                for idx in range(len(flat)):
                    if idx + LOOK < len(flat):
                        pend.append(issue_s(idx + LOOK))
                    pt, c0 = pend.pop(0)
                    issue_po(idx, pt, c0)

                ohs_prev = ohs
                t0_prev = t0

            # ---- final block's Wo ----
            for ts_ in range(4):
                for e in range(4):
                    y_ps = pss.tile([P, TB], F32, tag="big", bufs=6,
                                    name=f"ypsF_{ts_}_{e}")
                    for h in range(NREP):
                        nc.tensor.matmul(
                            y_ps, ohs_prev[h][:, ts_ * P:(ts_ + 1) * P],
                            wo_sb[:, h, e * TB:(e + 1) * TB],
                            start=(h == 0), stop=(h == NREP - 1))
                    y_sb = ypool.tile([P, TB], BF16, tag="ysb", name=f"yF_{ts_}_{e}")
                    nc.vector.tensor_copy(y_sb, y_ps)
                    nc.gpsimd.dma_start(
                        out=out[t0_prev + ts_ * P:t0_prev + (ts_ + 1) * P,
                                e * TB:(e + 1) * TB],
                        in_=y_sb)

    nc.compile()
    return nc


def _prep_inputs(x, mask, freqs_cis, alibi_bias, Wq, Wk, Wv, Wo):
    """Host-side prep: transposes, RoPE tables, ALiBi bias decomposition."""
    f64 = np.float64
    idx = np.arange(HD)
    cos_full = freqs_cis[:, idx // 2]                     # [T, 128]
    sin_full = freqs_cis[:, (HD // 2) + idx // 2]         # [T, 128]
    sign = np.where(idx < HD // 2, -1.0, 1.0).astype(np.float32)
    cosT = np.ascontiguousarray(cos_full.T)               # [128, T]
    sinT_signed = np.ascontiguousarray((sin_full * sign[None, :]).T)

    cosq = (cosT * np.float32(SCALE)).astype(BF)
    sinq = (sinT_signed * np.float32(SCALE)).astype(BF)
    cosk = cosT.astype(BF)
    sink = sinT_signed.astype(BF)

    # triangle mask block: key p > query c -> -1e9 (transposed layout)
    pp = np.arange(P)
    maskT = np.where(pp[:, None] > pp[None, :], -1e9, 0.0).astype(np.float32)

    onesc = np.ones((P, 1), BF)
    idin = np.eye(P, dtype=np.float32).astype(BF)
    permi = np.zeros((P, P), np.float32)
    permi[(np.arange(P) + P // 2) % P, np.arange(P)] = 1.0
    permi = permi.astype(BF)

    in_maps = []
    for c in range(8):
        b, g = c // 4, c % 4
        slopes = np.array([-f64(alibi_bias[0, g * NREP + hl, 1, 0]) for hl in range(NREP)])
        pvec = np.arange(P, dtype=f64)
        jvec = np.arange(NJ, dtype=f64)
        # cb[p, h, bk, j] = ALIBI_W*slope*(j*128 + p) - ALIBI_W*slope*(bk*512 + 511)
        bkvec = np.arange(NBLK, dtype=f64)
        cbv = (ALIBI_W * slopes[:, None, None, None]
               * (jvec[None, None, :, None] * P + pvec[None, None, None, :]
                  - (bkvec[None, :, None, None] * TB + (TB - 1))))
        cbm = np.ascontiguousarray(cbv.transpose(3, 0, 1, 2).reshape(P, NREP * NBLK * NJ)).astype(np.float32)
        in_maps.append({
            "xT": np.ascontiguousarray(x[b].T).astype(BF),
            "wq": np.ascontiguousarray(Wq[:, g * KVD:(g + 1) * KVD]).astype(BF),
            "wk": np.ascontiguousarray(Wk[:, g * P:(g + 1) * P]).astype(BF),
            "wv": np.ascontiguousarray(Wv[:, g * P:(g + 1) * P]).astype(BF),
            "wo": np.ascontiguousarray(Wo[g * KVD:(g + 1) * KVD, :]).astype(BF),
            "cosq": cosq, "sinq": sinq, "cosk": cosk, "sink": sink,
            "cb": cbm, "maskT": maskT,
            "onesc": onesc, "idin": idin, "permi": permi,
        })
    return in_maps


def kernel(x, mask, freqs_cis, alibi_bias, Wq, Wk, Wv, Wo, _trace=False, _trace_kwargs=None):
    from concourse.bass_utils import run_bass_kernel_spmd

    if "nc" not in _cache:
        _cache["nc"] = _build()
    nc = _cache["nc"]

    in_maps = _prep_inputs(np.asarray(x, np.float32), np.asarray(mask, np.float32),
                           np.asarray(freqs_cis, np.float32), np.asarray(alibi_bias, np.float32),
                           np.asarray(Wq, np.float32), np.asarray(Wk, np.float32),
                           np.asarray(Wv, np.float32), np.asarray(Wo, np.float32))
    kw = {}
    if _trace:
        kw = dict(trace=True, **(_trace_kwargs or {}))
    res = run_bass_kernel_spmd(nc, in_maps, list(range(8)), **kw)

    full = np.zeros((B, T, D), np.float32)
    for c in range(8):
        full[c // 4] += np.asarray(res.results[c]["out"], np.float32)
    if _trace:
        _cache["last_trace"] = res
    return full
